# revision 2
# baseline (speedup 1.0000x reference)
"""HardMoE classifier forward on 8 Trainium2 NeuronCores (Bass/Tile), v3.

Math (per row b of cls_token [B, D]):
    logits[j]  = cls_token[b] . Wcat[j],  j in 0..17
                 (Wcat = concat(gate_w [6,D], expert_w.reshape(12, D)))
    choice     = argmax(logits[0:6] + gate_b)      (first-index tiebreak)
    out[b, l]  = logits[6 + 2*choice + l] + expert_b[choice, l]

v3 design, driven by measured stage rates on this toolchain:
  * The PE transpose of x (needed to put the contraction dim on SBUF
    partitions) ran at ~206 ns per [128,128] chunk (transpose-mode does
    not engage the PE clock ramp) and its PSUM->SBUF copies loaded the
    vector engines: together ~40% of kernel time. v3 removes the whole
    stage: the HOST pre-transposes x and splits it into fp16 hi/lo
    planes (hi = fp16(x), lo = fp16(x - hi)) laid out per-super as
    [128 dpart, KC, 2048 cols]. Same DMA bytes as fp32 (2 x 2B), clean
    4 MB transfers (32 KB contiguous per partition).
  * Logits are computed as Whi.hi + Whi.lo + Wlo.hi in three fp16
    matmul passes (1 cyc/row vs fp32's 4) accumulating in fp32 PSUM.
    Validated host-side and on HW: max logit error ~2e-6, ~1 gate
    argmax flip per 131072 rows (rel-L2 contribution ~4e-3, gate 2e-2).
    Column-tiled matmuls (tile_position=(0,32j)) execute serially on
    this toolchain (LDWEIGHTS row-group conflict), so 3 fp16 passes
    (~20.5 us/super) beat 1 fp32 pass (~27.3 us/super).
  * Per 2048-row super: DMA 8 MB (hi+lo) ~25 us | PE 96 mms + 16
    back-transposes ~22-26 us | DVE/ACT select ~3 us. DMA and PE
    overlap; expect ~26-30 us/super, 8 supers/core.
"""

import json

import numpy as np

import concourse.bass as bass
import concourse.mybir as mybir
from concourse.bass_utils import run_bass_kernel_spmd
from concourse.tile import TileContext

F32 = mybir.dt.float32
F16 = mybir.dt.float16
ALU = mybir.AluOpType
AX = mybir.AxisListType

B, D, E, L = 131072, 1024, 6, 2
NCORES = 8
BLOC = B // NCORES            # 16384 rows per core
NJ = E + E * L                # 18 logit columns (6 gate + 12 expert)
KC = D // 128                 # 8 contraction chunks
NBLK = 4                      # 512-col mm blocks per super
SUP = NBLK * 512              # 2048 rows per super-iteration
NSUP = BLOC // SUP            # 8 super-iterations per core

# ---------------------------------------------------------------------------
# Workaround: this walrus build supports only ONE sync wait per instruction,
# but Tile emits instructions (and its tail drain) with several. Split the
# extra monotonic (sem-ge) waits onto single-wait NoOps placed immediately
# before the instruction on the same engine.
# ---------------------------------------------------------------------------
_wsplit_counter = [0]


def _split_multiwaits(mod: dict) -> dict:
    for fn in mod.get("functions", []):
        for blk in fn.get("blocks", []):
            out = []
            changed = False
            for ins in blk.get("instructions", []):
                si = ins.get("sync_info") or {}
                waits = si.get("on_wait") or []
                if len(waits) > 1:
                    changed = True
                    ge = [w for w in waits if w.get("wait_mode", "").startswith("sem-ge")]
                    rest = [w for w in waits if not w.get("wait_mode", "").startswith("sem-ge")]
                    assert len(rest) <= 1, (
                        f"multiple non-monotonic waits on {ins.get('name')}: {rest}"
                    )
                    keep = rest[0] if rest else ge.pop()
                    for w in ge:
                        _wsplit_counter[0] += 1
                        out.append({
                            "debug": ins.get("debug", 0),
                            "engine": ins["engine"],
                            "ins": [],
                            "name": f"WSPLIT-{_wsplit_counter[0]}",
                            "opcode": "NoOp",
                            "outs": [],
                            "sync_info": {"on_update": [], "on_wait": [w]},
                        })
                    si["on_wait"] = [keep]
                    ins["sync_info"] = si
                out.append(ins)
            if changed:
                blk["instructions"] = out
    return mod


_orig_to_json_bytes = bass.Bass.to_json_bytes


def _patched_to_json_bytes(self) -> bytes:
    mod = json.loads(_orig_to_json_bytes(self))
    return json.dumps(_split_multiwaits(mod)).encode()


if bass.Bass.to_json_bytes is not _patched_to_json_bytes:
    bass.Bass.to_json_bytes = _patched_to_json_bytes


# ---------------------------------------------------------------------------
# Device kernel (one NeuronCore's shard)
# ---------------------------------------------------------------------------

def _build_nc(time_loop: int = 0) -> bass.Bass:
    nc = bass.Bass(name="hardmoe3")
    # pre-transposed fp16 planes: [s, p, k*2048+c] = plane[2048 s + c, 128 k + p]
    xh = nc.dram_tensor("xh", [NSUP, 128, KC * SUP], F16, kind="ExternalInput")
    xl = nc.dram_tensor("xl", [NSUP, 128, KC * SUP], F16, kind="ExternalInput")
    wth = nc.dram_tensor("wth", [KC, 128, NJ], F16, kind="ExternalInput")
    wtl = nc.dram_tensor("wtl", [KC, 128, NJ], F16, kind="ExternalInput")
    bias = nc.dram_tensor("bias", [128, NJ], F32, kind="ExternalInput")
    desc = nc.dram_tensor("desc", [128, E], F32, kind="ExternalInput")
    idt32 = nc.dram_tensor("idt32", [128, NJ], F32, kind="ExternalInput")
    out = nc.dram_tensor("out", [BLOC, L], F32, kind="ExternalOutput")

    xhv = xh.rearrange("s p (k c) -> s p k c", k=KC)
    xlv = xl.rearrange("s p (k c) -> s p k c", k=KC)

    with TileContext(nc) as tc:
        with tc.tile_pool(name="const", bufs=1) as cpool, \
             tc.tile_pool(name="xt", bufs=2) as xtpool, \
             tc.tile_pool(name="psmm", bufs=1, space="PSUM") as psmm_pool, \
             tc.tile_pool(name="pstb", bufs=2, space="PSUM") as pstb_pool, \
             tc.tile_pool(name="lsb", bufs=2) as lpool, \
             tc.tile_pool(name="sel", bufs=2) as selpool:

            wth_sb = cpool.tile([128, KC, NJ], F16)
            nc.sync.dma_start(wth_sb[:], wth.rearrange("k p j -> p k j"))
            wtl_sb = cpool.tile([128, KC, NJ], F16)
            nc.sync.dma_start(wtl_sb[:], wtl.rearrange("k p j -> p k j"))
            bias_sb = cpool.tile([128, NJ], F32)
            nc.sync.dma_start(bias_sb[:], bias[:])
            desc_sb = cpool.tile([128, E], F32)
            nc.sync.dma_start(desc_sb[:], desc[:])
            ident32 = cpool.tile([128, NJ], F32)
            nc.sync.dma_start(ident32[:], idt32[:])

            def body():
                def stage_super(s: int):
                    hi = xtpool.tile([128, KC, SUP], F16, tag="hi")
                    nc.sync.dma_start(hi[:], xhv[s])
                    lo = xtpool.tile([128, KC, SUP], F16, tag="lo")
                    nc.sync.dma_start(lo[:], xlv[s])
                    return hi, lo

                live = {0: stage_super(0)}

                for s in range(NSUP):
                    if s + 1 < NSUP:
                        live[s + 1] = stage_super(s + 1)
                    hi, lo = live.pop(s)

                    # 3-term fp16 logitsT, col group j <-> psum rows [32j,32j+18)
                    ps_mm = [
                        psmm_pool.tile([128, 512], F32, tag=f"ps_mm{j}",
                                       name=f"ps_mm{j}")
                        for j in range(NBLK)
                    ]
                    for k in range(KC):
                        for j in range(NBLK):
                            o = ps_mm[j][32 * j:32 * j + NJ, :]
                            blk = slice(512 * j, 512 * (j + 1))
                            nc.tensor.matmul(
                                o, wth_sb[:, k], hi[:, k, blk],
                                start=(k == 0), stop=False,
                                tile_position=(0, 32 * j),
                            )
                            nc.tensor.matmul(
                                o, wth_sb[:, k], lo[:, k, blk],
                                start=False, stop=False,
                                tile_position=(0, 32 * j),
                            )
                            nc.tensor.matmul(
                                o, wtl_sb[:, k], hi[:, k, blk],
                                start=False, stop=(k == KC - 1),
                                tile_position=(0, 32 * j),
                            )
                    l_sb = lpool.tile([128, 512], F32, tag="l_sb")
                    for j in range(NBLK):
                        nc.scalar.copy(
                            l_sb[32 * j:32 * j + NJ, :],
                            ps_mm[j][32 * j:32 * j + NJ, :],
                        )

                    # back-transpose logitsT strips to [rows, 18] and select;
                    # column c of super s is row 2048 s + c
                    for mg in range(2):
                        tp = pstb_pool.tile([128, 8, NJ], F32, tag="tp")
                        for half in range(8):
                            j = mg * 2 + half // 4
                            c = half % 4
                            nc.tensor.matmul(
                                tp[:, half, :],
                                l_sb[32 * j:32 * j + NJ, c * 128:(c + 1) * 128],
                                ident32[32 * j:32 * j + NJ, :],
                                is_transpose=True,
                                tile_position=(32 * j, 0),
                            )
                        A = selpool.tile([128, 8, NJ], F32, tag="A")
                        nc.scalar.copy(A[:], tp[:])
                        nc.vector.tensor_tensor(
                            A[:], A[:],
                            bias_sb[:, None, :].to_broadcast([128, 8, NJ]),
                            ALU.add,
                        )
                        gate = A[:, :, 0:E]
                        m = selpool.tile([128, 8], F32, tag="m")
                        nc.vector.tensor_reduce(m[:], gate, AX.X, ALU.max)
                        eq = selpool.tile([128, 8, E], F32, tag="eq")
                        nc.vector.tensor_tensor(
                            eq[:], gate, m[:, :, None].to_broadcast([128, 8, E]),
                            ALU.is_ge,
                        )
                        nc.vector.tensor_tensor(
                            eq[:], eq[:],
                            desc_sb[:, None, :].to_broadcast([128, 8, E]),
                            ALU.mult,
                        )
                        nc.vector.tensor_reduce(m[:], eq[:], AX.X, ALU.max)
                        onehot = selpool.tile([128, 8, E], F32, tag="onehot")
                        nc.vector.tensor_tensor(
                            onehot[:], eq[:], m[:, :, None].to_broadcast([128, 8, E]),
                            ALU.is_equal,
                        )
                        outs = selpool.tile([128, 8, L], F32, tag="outs")
                        sel = selpool.tile([128, 8, E], F32, tag="sel")
                        for l in range(L):
                            nc.vector.tensor_tensor(
                                sel[:], onehot[:], A[:, :, E + l::L], ALU.mult
                            )
                            nc.vector.tensor_reduce(
                                outs[:, :, l], sel[:], AX.X, ALU.add
                            )
                        r0 = (2 * s + mg) * 1024
                        nc.sync.dma_start(
                            out[r0:r0 + 1024, :].rearrange("(g p) l -> p g l", p=128),
                            outs[:],
                        )

            if time_loop:
                with tc.For_i(0, time_loop, 1, name="timing") as _i:
                    body()
            else:
                body()
    return nc


_cached = None


def _get_nc() -> bass.Bass:
    global _cached
    if _cached is None:
        _cached = _build_nc()
    return _cached


# ---------------------------------------------------------------------------
# Host wrapper
# ---------------------------------------------------------------------------

def _host_inputs(cls_token, gate_w, gate_b, expert_w, expert_b):
    x = np.asarray(cls_token, dtype=np.float32)
    gw = np.asarray(gate_w, dtype=np.float32)
    gb = np.asarray(gate_b, dtype=np.float32)
    ew = np.asarray(expert_w, dtype=np.float32)
    eb = np.asarray(expert_b, dtype=np.float32)
    assert x.shape == (B, D), x.shape

    hi = x.astype(np.float16)
    lo = (x - hi.astype(np.float32)).astype(np.float16)

    def tile_plane(p):
        # per core: [BLOC, D] -> [NSUP, 128, KC*SUP] with
        # [s, q, k*SUP + c] = plane[2048 s + c, 128 k + q]
        t = p.reshape(NCORES, NSUP, SUP, KC, 128)
        return np.ascontiguousarray(t.transpose(0, 1, 4, 3, 2)).reshape(
            NCORES, NSUP, 128, KC * SUP)

    hi_t = tile_plane(hi)
    lo_t = tile_plane(lo)

    wcat = np.concatenate([gw, ew.reshape(E * L, D)], axis=0)      # [18, D]
    wh = wcat.astype(np.float16)
    wl = (wcat - wh.astype(np.float32)).astype(np.float16)
    wth_in = np.ascontiguousarray(wh.T).reshape(KC, 128, NJ)
    wtl_in = np.ascontiguousarray(wl.T).reshape(KC, 128, NJ)
    bias_in = np.ascontiguousarray(np.broadcast_to(
        np.concatenate([gb, eb.reshape(E * L)])[None, :], (128, NJ)))
    desc_in = np.ascontiguousarray(np.broadcast_to(
        (E - np.arange(E, dtype=np.float32))[None, :], (128, E)))
    idt32_in = np.zeros((128, NJ), np.float32)
    for p in range(128):
        if p % 32 < NJ:
            idt32_in[p, p % 32] = 1.0

    in_maps = []
    for c in range(NCORES):
        in_maps.append({
            "xh": hi_t[c],
            "xl": lo_t[c],
            "wth": wth_in,
            "wtl": wtl_in,
            "bias": bias_in,
            "desc": desc_in,
            "idt32": idt32_in,
        })
    return in_maps


def kernel(cls_token, gate_w, gate_b, expert_w, expert_b) -> np.ndarray:
    in_maps = _host_inputs(cls_token, gate_w, gate_b, expert_w, expert_b)
    res = run_bass_kernel_spmd(_get_nc(), in_maps, core_ids=list(range(NCORES)))
    return np.concatenate([r["out"] for r in res.results], axis=0)


# revision 3
# speedup vs baseline: 1.0866x; 1.0866x over previous
"""HardMoE classifier forward on 8 Trainium2 NeuronCores (Bass/Tile), v3.

Math (per row b of cls_token [B, D]):
    logits[j]  = cls_token[b] . Wcat[j],  j in 0..17
                 (Wcat = concat(gate_w [6,D], expert_w.reshape(12, D)))
    choice     = argmax(logits[0:6] + gate_b)      (first-index tiebreak)
    out[b, l]  = logits[6 + 2*choice + l] + expert_b[choice, l]

v3 design, driven by measured stage rates on this toolchain:
  * The PE transpose of x (needed to put the contraction dim on SBUF
    partitions) ran at ~206 ns per [128,128] chunk (transpose-mode does
    not engage the PE clock ramp) and its PSUM->SBUF copies loaded the
    vector engines: together ~40% of kernel time. v3 removes the whole
    stage: the HOST pre-transposes x and splits it into fp16 hi/lo
    planes (hi = fp16(x), lo = fp16(x - hi)) laid out per-super as
    [128 dpart, KC, 2048 cols]. Same DMA bytes as fp32 (2 x 2B), clean
    4 MB transfers (32 KB contiguous per partition).
  * Logits are computed as Whi.hi + Whi.lo + Wlo.hi in three fp16
    matmul passes (1 cyc/row vs fp32's 4) accumulating in fp32 PSUM.
    Validated host-side and on HW: max logit error ~2e-6, ~1 gate
    argmax flip per 131072 rows (rel-L2 contribution ~4e-3, gate 2e-2).
    Column-tiled matmuls (tile_position=(0,32j)) execute serially on
    this toolchain (LDWEIGHTS row-group conflict), so 3 fp16 passes
    (~20.5 us/super) beat 1 fp32 pass (~27.3 us/super).
  * Per 2048-row super: DMA 8 MB (hi+lo) ~25 us | PE 96 mms + 16
    back-transposes ~22-26 us | DVE/ACT select ~3 us. DMA and PE
    overlap; expect ~26-30 us/super, 8 supers/core.
"""

import json

import numpy as np

import concourse.bass as bass
import concourse.mybir as mybir
from concourse.bass_utils import run_bass_kernel_spmd
from concourse.tile import TileContext

F32 = mybir.dt.float32
F16 = mybir.dt.float16
ALU = mybir.AluOpType
AX = mybir.AxisListType

B, D, E, L = 131072, 1024, 6, 2
NCORES = 8
BLOC = B // NCORES            # 16384 rows per core
NJ = E + E * L                # 18 logit columns (6 gate + 12 expert)
KC = D // 128                 # 8 contraction chunks
NBLK = 4                      # 512-col mm blocks per super
SUP = NBLK * 512              # 2048 rows per super-iteration
NSUP = BLOC // SUP            # 8 super-iterations per core

# ---------------------------------------------------------------------------
# Workaround: this walrus build supports only ONE sync wait per instruction,
# but Tile emits instructions (and its tail drain) with several. Split the
# extra monotonic (sem-ge) waits onto single-wait NoOps placed immediately
# before the instruction on the same engine.
# ---------------------------------------------------------------------------
_wsplit_counter = [0]


def _split_multiwaits(mod: dict) -> dict:
    for fn in mod.get("functions", []):
        for blk in fn.get("blocks", []):
            out = []
            changed = False
            for ins in blk.get("instructions", []):
                si = ins.get("sync_info") or {}
                waits = si.get("on_wait") or []
                if len(waits) > 1:
                    changed = True
                    ge = [w for w in waits if w.get("wait_mode", "").startswith("sem-ge")]
                    rest = [w for w in waits if not w.get("wait_mode", "").startswith("sem-ge")]
                    assert len(rest) <= 1, (
                        f"multiple non-monotonic waits on {ins.get('name')}: {rest}"
                    )
                    keep = rest[0] if rest else ge.pop()
                    for w in ge:
                        _wsplit_counter[0] += 1
                        out.append({
                            "debug": ins.get("debug", 0),
                            "engine": ins["engine"],
                            "ins": [],
                            "name": f"WSPLIT-{_wsplit_counter[0]}",
                            "opcode": "NoOp",
                            "outs": [],
                            "sync_info": {"on_update": [], "on_wait": [w]},
                        })
                    si["on_wait"] = [keep]
                    ins["sync_info"] = si
                out.append(ins)
            if changed:
                blk["instructions"] = out
    return mod


_orig_to_json_bytes = bass.Bass.to_json_bytes


def _patched_to_json_bytes(self) -> bytes:
    mod = json.loads(_orig_to_json_bytes(self))
    return json.dumps(_split_multiwaits(mod)).encode()


if bass.Bass.to_json_bytes is not _patched_to_json_bytes:
    bass.Bass.to_json_bytes = _patched_to_json_bytes


# ---------------------------------------------------------------------------
# Device kernel (one NeuronCore's shard)
# ---------------------------------------------------------------------------

def _build_nc(time_loop: int = 0) -> bass.Bass:
    nc = bass.Bass(name="hardmoe4")
    # pre-transposed fp16 hi/lo planes, combined per super into one 8 MB
    # transfer: [s, p, (h k c)] with h=0 the hi plane, h=1 the lo plane,
    # [s, p, h, k, c] = plane_h[2048 s + c, 128 k + p]
    xhl = nc.dram_tensor("xhl", [NSUP, 128, 2 * KC * SUP], F16,
                         kind="ExternalInput")
    wth = nc.dram_tensor("wth", [KC, 128, NJ], F16, kind="ExternalInput")
    wtl = nc.dram_tensor("wtl", [KC, 128, NJ], F16, kind="ExternalInput")
    bias = nc.dram_tensor("bias", [128, NJ], F32, kind="ExternalInput")
    desc = nc.dram_tensor("desc", [128, E], F32, kind="ExternalInput")
    idt32 = nc.dram_tensor("idt32", [128, NJ], F32, kind="ExternalInput")
    out = nc.dram_tensor("out", [BLOC, L], F32, kind="ExternalOutput")

    with TileContext(nc) as tc:
        with tc.tile_pool(name="const", bufs=1) as cpool, \
             tc.tile_pool(name="xt", bufs=3) as xtpool, \
             tc.tile_pool(name="psmm", bufs=1, space="PSUM") as psmm_pool, \
             tc.tile_pool(name="pstb", bufs=2, space="PSUM") as pstb_pool, \
             tc.tile_pool(name="lsb", bufs=2) as lpool, \
             tc.tile_pool(name="sel", bufs=2) as selpool:

            wth_sb = cpool.tile([128, KC, NJ], F16)
            nc.sync.dma_start(wth_sb[:], wth.rearrange("k p j -> p k j"))
            wtl_sb = cpool.tile([128, KC, NJ], F16)
            nc.sync.dma_start(wtl_sb[:], wtl.rearrange("k p j -> p k j"))
            bias_sb = cpool.tile([128, NJ], F32)
            nc.sync.dma_start(bias_sb[:], bias[:])
            desc_sb = cpool.tile([128, E], F32)
            nc.sync.dma_start(desc_sb[:], desc[:])
            ident32 = cpool.tile([128, NJ], F32)
            nc.sync.dma_start(ident32[:], idt32[:])

            def body():
                def stage_super(s: int):
                    hl = xtpool.tile([128, 2, KC, SUP], F16, tag="hl")
                    nc.sync.dma_start(hl[:], xhl[s])
                    return hl

                live = {0: stage_super(0)}
                for s in range(1, min(2, NSUP)):
                    live[s] = stage_super(s)

                for s in range(NSUP):
                    if s + 2 < NSUP:
                        live[s + 2] = stage_super(s + 2)
                    hl = live.pop(s)
                    hi = hl[:, 0]
                    lo = hl[:, 1]

                    # 3-term fp16 logitsT, col group j <-> psum rows [32j,32j+18)
                    ps_mm = [
                        psmm_pool.tile([128, 512], F32, tag=f"ps_mm{j}",
                                       name=f"ps_mm{j}")
                        for j in range(NBLK)
                    ]
                    for k in range(KC):
                        for j in range(NBLK):
                            o = ps_mm[j][32 * j:32 * j + NJ, :]
                            blk = slice(512 * j, 512 * (j + 1))
                            nc.tensor.matmul(
                                o, wth_sb[:, k], hi[:, k, blk],
                                start=(k == 0), stop=False,
                                tile_position=(0, 32 * j),
                            )
                            nc.tensor.matmul(
                                o, wth_sb[:, k], lo[:, k, blk],
                                start=False, stop=False,
                                tile_position=(0, 32 * j),
                            )
                            nc.tensor.matmul(
                                o, wtl_sb[:, k], hi[:, k, blk],
                                start=False, stop=(k == KC - 1),
                                tile_position=(0, 32 * j),
                            )
                    l_sb = lpool.tile([128, 512], F32, tag="l_sb")
                    for j in range(NBLK):
                        nc.scalar.copy(
                            l_sb[32 * j:32 * j + NJ, :],
                            ps_mm[j][32 * j:32 * j + NJ, :],
                        )

                    # back-transpose logitsT strips to [rows, 18] and select;
                    # column c of super s is row 2048 s + c
                    for mg in range(2):
                        tp = pstb_pool.tile([128, 8, NJ], F32, tag="tp")
                        for half in range(8):
                            j = mg * 2 + half // 4
                            c = half % 4
                            nc.tensor.matmul(
                                tp[:, half, :],
                                l_sb[32 * j:32 * j + NJ, c * 128:(c + 1) * 128],
                                ident32[32 * j:32 * j + NJ, :],
                                is_transpose=True,
                                tile_position=(32 * j, 0),
                            )
                        A = selpool.tile([128, 8, NJ], F32, tag="A")
                        nc.scalar.copy(A[:], tp[:])
                        nc.vector.tensor_tensor(
                            A[:], A[:],
                            bias_sb[:, None, :].to_broadcast([128, 8, NJ]),
                            ALU.add,
                        )
                        gate = A[:, :, 0:E]
                        m = selpool.tile([128, 8], F32, tag="m")
                        nc.vector.tensor_reduce(m[:], gate, AX.X, ALU.max)
                        eq = selpool.tile([128, 8, E], F32, tag="eq")
                        nc.vector.tensor_tensor(
                            eq[:], gate, m[:, :, None].to_broadcast([128, 8, E]),
                            ALU.is_ge,
                        )
                        nc.vector.tensor_tensor(
                            eq[:], eq[:],
                            desc_sb[:, None, :].to_broadcast([128, 8, E]),
                            ALU.mult,
                        )
                        nc.vector.tensor_reduce(m[:], eq[:], AX.X, ALU.max)
                        onehot = selpool.tile([128, 8, E], F32, tag="onehot")
                        nc.vector.tensor_tensor(
                            onehot[:], eq[:], m[:, :, None].to_broadcast([128, 8, E]),
                            ALU.is_equal,
                        )
                        outs = selpool.tile([128, 8, L], F32, tag="outs")
                        sel = selpool.tile([128, 8, E], F32, tag="sel")
                        for l in range(L):
                            nc.vector.tensor_tensor(
                                sel[:], onehot[:], A[:, :, E + l::L], ALU.mult
                            )
                            nc.vector.tensor_reduce(
                                outs[:, :, l], sel[:], AX.X, ALU.add
                            )
                        r0 = (2 * s + mg) * 1024
                        nc.scalar.dma_start(
                            out[r0:r0 + 1024, :].rearrange("(g p) l -> p g l", p=128),
                            outs[:],
                        )

            if time_loop:
                with tc.For_i(0, time_loop, 1, name="timing") as _i:
                    body()
            else:
                body()
    return nc


_cached = None


def _get_nc() -> bass.Bass:
    global _cached
    if _cached is None:
        _cached = _build_nc()
    return _cached


# ---------------------------------------------------------------------------
# Host wrapper
# ---------------------------------------------------------------------------

def _host_inputs(cls_token, gate_w, gate_b, expert_w, expert_b):
    x = np.asarray(cls_token, dtype=np.float32)
    gw = np.asarray(gate_w, dtype=np.float32)
    gb = np.asarray(gate_b, dtype=np.float32)
    ew = np.asarray(expert_w, dtype=np.float32)
    eb = np.asarray(expert_b, dtype=np.float32)
    assert x.shape == (B, D), x.shape

    hi = x.astype(np.float16)
    lo = (x - hi.astype(np.float32)).astype(np.float16)

    def tile_plane(p):
        # per core: [BLOC, D] -> [NSUP, 128, KC, SUP] with
        # [s, q, k, c] = plane[2048 s + c, 128 k + q]
        t = p.reshape(NCORES, NSUP, SUP, KC, 128)
        return t.transpose(0, 1, 4, 3, 2)

    # combined per-super 8 MB block: [core, s, q, h, k, c]
    hl_t = np.ascontiguousarray(
        np.stack([tile_plane(hi), tile_plane(lo)], axis=3)).reshape(
            NCORES, NSUP, 128, 2 * KC * SUP)

    wcat = np.concatenate([gw, ew.reshape(E * L, D)], axis=0)      # [18, D]
    wh = wcat.astype(np.float16)
    wl = (wcat - wh.astype(np.float32)).astype(np.float16)
    wth_in = np.ascontiguousarray(wh.T).reshape(KC, 128, NJ)
    wtl_in = np.ascontiguousarray(wl.T).reshape(KC, 128, NJ)
    bias_in = np.ascontiguousarray(np.broadcast_to(
        np.concatenate([gb, eb.reshape(E * L)])[None, :], (128, NJ)))
    desc_in = np.ascontiguousarray(np.broadcast_to(
        (E - np.arange(E, dtype=np.float32))[None, :], (128, E)))
    idt32_in = np.zeros((128, NJ), np.float32)
    for p in range(128):
        if p % 32 < NJ:
            idt32_in[p, p % 32] = 1.0

    in_maps = []
    for c in range(NCORES):
        in_maps.append({
            "xhl": hl_t[c],
            "wth": wth_in,
            "wtl": wtl_in,
            "bias": bias_in,
            "desc": desc_in,
            "idt32": idt32_in,
        })
    return in_maps


def kernel(cls_token, gate_w, gate_b, expert_w, expert_b) -> np.ndarray:
    in_maps = _host_inputs(cls_token, gate_w, gate_b, expert_w, expert_b)
    res = run_bass_kernel_spmd(_get_nc(), in_maps, core_ids=list(range(NCORES)))
    return np.concatenate([r["out"] for r in res.results], axis=0)
